# revision 4
# baseline (speedup 1.0000x reference)
"""Trainium2 Bass kernel for nn_KinematicWaveRouting.

Math: the reference runs a lax.scan over T=4096 steps of
    Q_new[i] = max(Q[i] - CFL*(Q[i] - Q[i-1]) + q_in*DT, 0),  i = 1..20, Q[0] = 0
with CFL = 0.9 and q_in >= 0. Every term is nonnegative, so the max never
clips and the recurrence is linear time-invariant. The outlet (segment 20)
is therefore an exact causal FIR filter of the scaled runoff:

    outlet[b, t] = sum_{k=0}^{K-1} h[k] * u[b, t-k]
    u[b, t]      = runoff[b, t] * basin_area[b] * 50
    h[k]         = P(Binom(k, 0.9) <= 19)   (== 1 for k < 20, ~0 for k > 36)

K = 40 taps reproduces the reference to the f32 rounding floor
(L2 rel err ~1.5e-7, max elementwise ~8e-7 measured offline).

Kernel layout (per core, batch shard of 1024 rows):
  - host passes runoff.T shard (4096, 1024) so time lies on SBUF partitions
  - out(b, t-block) = lhsT.T @ rhs with lhsT = x^T chunk (stationary,
    contraction = time) and rhs = constant banded Toeplitz tap matrices
    A0[s,t] = h[t-s], A1[s,t] = h[t+128-s]; output lands directly in
    natural (batch, time) layout in PSUM
  - per-row scale (basin_area*50) fused into the PSUM->SBUF copy on DVE
All matmuls in fp32 (4-pass PE) to keep full precision.
"""

import math

import numpy as np

import concourse.bacc as bacc
import concourse.bass as bass
import concourse.mybir as mybir
import concourse.tile as tile
from concourse.bass_utils import run_bass_kernel_spmd

N_CORES = 8
B_FULL, T = 8192, 4096
BSH = B_FULL // N_CORES          # 1024 rows per core
NSEG = 20
CFL = float(np.float32(0.9))
K_TAPS = 40
CHUNK = 128
NCHUNK = T // CHUNK              # 32
RG = BSH // 128                  # 8 row groups per core
JGRP = 4                         # chunks per PSUM bank (4*128 = 512 f32 = 1 bank)
F32 = mybir.dt.float32


def _taps() -> np.ndarray:
    """h[k] = P(Binom(k, CFL) <= NSEG-1), computed exactly in f64."""
    c, a = CFL, 1.0 - CFL
    h = np.zeros(K_TAPS, dtype=np.float64)
    for k in range(K_TAPS):
        h[k] = sum(math.comb(k, m) * c**m * a ** (k - m)
                   for m in range(0, min(k, NSEG - 1) + 1))
    return h


def _tap_matrices() -> tuple[np.ndarray, np.ndarray]:
    h = _taps()
    a0 = np.zeros((CHUNK, CHUNK), dtype=np.float32)
    for s in range(CHUNK):
        for t in range(s, min(s + K_TAPS, CHUNK)):
            a0[s, t] = h[t - s]
    a1 = np.zeros((CHUNK, K_TAPS - 1), dtype=np.float32)
    for t in range(K_TAPS - 1):
        for s in range(t + CHUNK - K_TAPS + 1, CHUNK):
            a1[s, t] = h[t + CHUNK - s]
    return a0, a1


def _build_nc() -> bass.Bass:
    # Bacc (not raw Bass): its compile() runs move_matmul_waits_to_ldweights +
    # generate_event_semaphores, which split >1-wait instructions into the
    # form TRN2 codegen accepts ("Too many sync wait commands" otherwise).
    nc = bacc.Bacc(None, target_bir_lowering=False)
    xT = nc.dram_tensor("xT", [T, BSH], F32, kind="ExternalInput")
    scale = nc.dram_tensor("scale", [CHUNK, RG], F32, kind="ExternalInput")
    a0 = nc.dram_tensor("a0", [CHUNK, CHUNK], F32, kind="ExternalInput")
    a1 = nc.dram_tensor("a1", [CHUNK, K_TAPS - 1], F32, kind="ExternalInput")
    out = nc.dram_tensor("out", [BSH, T], F32, kind="ExternalOutput")

    with tile.TileContext(nc) as tc:
        with (
            tc.tile_pool(name="consts", bufs=1) as consts,
            tc.tile_pool(name="xp", bufs=1) as xp,
            tc.tile_pool(name="op", bufs=3) as op,
            tc.tile_pool(name="psp", bufs=4, space="PSUM") as psp,
        ):
            a0_sb = consts.tile([CHUNK, CHUNK], F32)
            nc.sync.dma_start(out=a0_sb, in_=a0[:, :])
            a1_sb = consts.tile([CHUNK, K_TAPS - 1], F32)
            nc.sync.dma_start(out=a1_sb, in_=a1[:, :])
            sc_sb = consts.tile([CHUNK, RG], F32)
            nc.sync.dma_start(out=sc_sb, in_=scale[:, :])

            # All 32 time-chunks of the transposed shard stay SBUF-resident
            # (32 * 4 KiB/partition = 128 KiB/partition).
            xts = []
            for j in range(NCHUNK):
                xt = xp.tile([CHUNK, BSH], F32, tag=f"x{j}")
                nc.sync.dma_start(out=xt, in_=xT[j * CHUNK:(j + 1) * CHUNK, :])
                xts.append(xt)

            for rg in range(RG):
                cs = slice(rg * CHUNK, (rg + 1) * CHUNK)
                for jg in range(NCHUNK // JGRP):
                    ps = psp.tile([CHUNK, JGRP * CHUNK], F32, tag="ps")
                    for jj in range(JGRP):
                        j = jg * JGRP + jj
                        pslice = ps[:, jj * CHUNK:(jj + 1) * CHUNK]
                        if j == 0:
                            nc.tensor.matmul(pslice, xts[j][:, cs], a0_sb,
                                             start=True, stop=True)
                        else:
                            nc.tensor.matmul(pslice, xts[j][:, cs], a0_sb,
                                             start=True, stop=False)
                            nc.tensor.matmul(
                                ps[:, jj * CHUNK:jj * CHUNK + K_TAPS - 1],
                                xts[j - 1][:, cs], a1_sb,
                                start=False, stop=True)
                    ot = op.tile([CHUNK, JGRP * CHUNK], F32, tag="o")
                    nc.vector.tensor_scalar_mul(ot, ps, sc_sb[:, rg:rg + 1])
                    nc.sync.dma_start(
                        out=out[cs, jg * JGRP * CHUNK:(jg + 1) * JGRP * CHUNK],
                        in_=ot)
    return nc


def _prep_inputs(runoff: np.ndarray, basin_area: np.ndarray):
    """Shard + layout prep on host. Returns per-core input maps."""
    runoff = np.ascontiguousarray(np.asarray(runoff, dtype=np.float32))
    basin_area = np.asarray(basin_area, dtype=np.float32).reshape(-1)
    scale_full = basin_area * np.float32(50.0)
    a0, a1 = _tap_matrices()
    in_maps = []
    for c in range(N_CORES):
        rows = slice(c * BSH, (c + 1) * BSH)
        xTc = np.ascontiguousarray(runoff[rows, :].T)          # (T, BSH)
        sc = np.ascontiguousarray(
            scale_full[rows].reshape(RG, CHUNK).T)             # (128, RG)
        in_maps.append({"xT": xTc, "scale": sc, "a0": a0, "a1": a1})
    return in_maps


def _run(inputs: dict, trace: bool = False):
    in_maps = _prep_inputs(inputs["runoff"], inputs["basin_area"])
    nc = _build_nc()
    # Bacc defers wait-splitting + register allocation to finalize();
    # run_bass_via_pjrt serializes nc.m as-is, so finalize here.
    nc.finalize()
    res = run_bass_kernel_spmd(nc, in_maps, core_ids=list(range(N_CORES)),
                               trace=trace)
    out = np.concatenate([m["out"] for m in res.results], axis=0)
    return out, res


def kernel(runoff, basin_area, manning_n=None, slope=None, width=None,
           **_unused):
    out, _ = _run({"runoff": runoff, "basin_area": basin_area})
    return out
